# revision 2
# baseline (speedup 1.0000x reference)
"""Binary-weight dense layer on 8 TRN2 NeuronCores.

Computes out = x @ sign(W) + b for x:[8192,4096] f32, W:[4096,4096] f32,
b:[4096] f32, sharded row-wise over x (tensor-parallel over the batch dim:
each core computes a [1024, 4096] slice of the output; no collectives).

Per-core kernel strategy:
  - x is split into x_hi = bf16(x) and x_lo = bf16(x - x_hi); the matmul is
    done in two bf16 passes accumulating into the same fp32 PSUM group, which
    recovers ~fp32 accuracy at 2x the bf16 matmul cost (sign(W) is exactly
    representable in bf16, so the only rounding is on x).
  - W tiles stream fp32 from HBM and are quantized on the scalar engine
    (Sign activation) into bf16 rhs tiles.
  - x_hi/x_lo are staged to DRAM and transposed back into SBUF-resident
    [K=128, M=1024] lhsT tiles via xbar DMA transpose (bf16-only path).
  - Main loop: for each n-slice (512 cols) keep 8 PSUM banks (one per m-tile)
    accumulating over all 32 k-tiles x {hi, lo}; evict with a DVE bias-add.
"""

import sys

if "/opt/trn_rl_repo" not in sys.path:
    sys.path.insert(0, "/opt/trn_rl_repo")

import numpy as np

import concourse.bass as bass
import concourse.mybir as mybir
import concourse.tile as tile
from concourse import bacc
from concourse.bass_utils import run_bass_kernel_spmd

N_CORES = 8
P = 128

B, N_IN, N_UNITS = 8192, 4096, 4096
M_SH = B // N_CORES  # 1024 rows of x per core

F32 = mybir.dt.float32
BF16 = mybir.dt.bfloat16


def build_module(m_sh=M_SH, k_dim=N_IN, n_dim=N_UNITS):
    """Build + compile the per-core Bass module (same program on all cores)."""
    nc = bacc.Bacc("TRN2", target_bir_lowering=False, debug=False)

    x_in = nc.dram_tensor("x", [m_sh, k_dim], F32, kind="ExternalInput")
    w_in = nc.dram_tensor("W", [k_dim, n_dim], F32, kind="ExternalInput")
    b_in = nc.dram_tensor("b", [n_dim], F32, kind="ExternalInput")
    out = nc.dram_tensor("out", [m_sh, n_dim], F32, kind="ExternalOutput")

    NT = 512  # psum free dim (one bank of fp32)
    KT = P  # contraction tile
    m_tiles = m_sh // P
    k_tiles = k_dim // KT
    n_slices = n_dim // NT
    PREP_C = min(2048, k_dim)  # prep chunk of the k axis (bounds prep SBUF usage)
    prep_chunks = k_dim // PREP_C

    with tile.TileContext(nc) as tc:
        with (
            tc.tile_pool(name="dram", bufs=1, space="DRAM") as dram,
            tc.tile_pool(name="xt", bufs=1) as xt_pool,
            tc.tile_pool(name="const", bufs=1) as const_pool,
        ):
            x_hi_dram = dram.tile([m_sh, k_dim], BF16)
            x_lo_dram = dram.tile([m_sh, k_dim], BF16)

            # SBUF-resident transposed activations: column block kt holds
            # [K=128, M=m_sh] for contraction tile kt.
            xt_hi = xt_pool.tile([P, k_tiles * m_sh], BF16)
            xt_lo = xt_pool.tile([P, k_tiles * m_sh], BF16)

            b_bc = const_pool.tile([P, n_dim], F32)
            nc.sync.dma_start(
                b_bc[:], b_in.ap().rearrange("(a n) -> a n", a=1).broadcast_to([P, n_dim])
            )

            # ---- Stage 1: split x into bf16 hi/lo, stage to DRAM ----
            with tc.tile_pool(name="prep", bufs=2) as prep:
                for mt in range(m_tiles):
                    for c in range(prep_chunks):
                        cs = slice(c * PREP_C, (c + 1) * PREP_C)
                        ms = slice(mt * P, (mt + 1) * P)
                        xin = prep.tile([P, PREP_C], F32)
                        nc.sync.dma_start(xin[:], x_in[ms, cs])
                        xhi = prep.tile([P, PREP_C], BF16)
                        nc.vector.tensor_copy(xhi[:], xin[:])
                        xhi_f = prep.tile([P, PREP_C], F32)
                        nc.scalar.copy(xhi_f[:], xhi[:])
                        xlo = prep.tile([P, PREP_C], BF16)
                        nc.vector.tensor_sub(xlo[:], xin[:], xhi_f[:])
                        nc.sync.dma_start(x_hi_dram[ms, cs], xhi[:])
                        nc.sync.dma_start(x_lo_dram[ms, cs], xlo[:])

            # ---- Stage 2: transpose hi/lo into SBUF lhsT layout ----
            for kt in range(k_tiles):
                ks = slice(kt * KT, (kt + 1) * KT)
                os_ = slice(kt * m_sh, (kt + 1) * m_sh)
                nc.sync.dma_start_transpose(xt_hi[:, os_], x_hi_dram[:, ks])
                nc.sync.dma_start_transpose(xt_lo[:, os_], x_lo_dram[:, ks])

            # ---- Stage 3: main matmul loop ----
            with (
                tc.tile_pool(name="wf", bufs=3) as wf_pool,
                tc.tile_pool(name="wq", bufs=3) as wq_pool,
                tc.tile_pool(name="psum", bufs=8, space="PSUM") as psum_pool,
                tc.tile_pool(name="osb", bufs=3) as out_pool,
            ):
                for ns in range(n_slices):
                    nss = slice(ns * NT, (ns + 1) * NT)
                    psums = []
                    for mt in range(m_tiles):
                        pt = psum_pool.tile([P, NT], F32, name=f"ps_{ns}_{mt}", tag="ps")
                        psums.append(pt)
                    for kt in range(k_tiles):
                        wf = wf_pool.tile([P, NT], F32)
                        nc.sync.dma_start(
                            wf[:], w_in[kt * KT : (kt + 1) * KT, nss]
                        )
                        wq = wq_pool.tile([P, NT], BF16)
                        nc.scalar.sign(wq[:], wf[:])
                        for mt in range(m_tiles):
                            lhs_o = kt * m_sh + mt * P
                            nc.tensor.matmul(
                                psums[mt][:],
                                xt_hi[:, lhs_o : lhs_o + P],
                                wq[:],
                                start=(kt == 0),
                                stop=False,
                            )
                            nc.tensor.matmul(
                                psums[mt][:],
                                xt_lo[:, lhs_o : lhs_o + P],
                                wq[:],
                                start=False,
                                stop=(kt == k_tiles - 1),
                            )
                    for mt in range(m_tiles):
                        osb = out_pool.tile([P, NT], F32)
                        nc.vector.tensor_add(osb[:], psums[mt][:], b_bc[:, nss])
                        nc.sync.dma_start(
                            out[mt * P : (mt + 1) * P, nss], osb[:]
                        )

    nc.compile()
    return nc


_NC_CACHE = {}


def _get_module(m_sh=M_SH, k_dim=N_IN, n_dim=N_UNITS):
    key = (m_sh, k_dim, n_dim)
    if key not in _NC_CACHE:
        _NC_CACHE[key] = build_module(m_sh, k_dim, n_dim)
    return _NC_CACHE[key]


def kernel(x: np.ndarray, W: np.ndarray, b: np.ndarray) -> np.ndarray:
    x = np.ascontiguousarray(np.asarray(x, dtype=np.float32))
    W = np.ascontiguousarray(np.asarray(W, dtype=np.float32))
    b = np.ascontiguousarray(np.asarray(b, dtype=np.float32))
    assert x.shape == (B, N_IN) and W.shape == (N_IN, N_UNITS) and b.shape == (N_UNITS,)

    nc = _get_module()
    in_maps = [
        {"x": x[i * M_SH : (i + 1) * M_SH], "W": W, "b": b} for i in range(N_CORES)
    ]
    res = run_bass_kernel_spmd(nc, in_maps, core_ids=list(range(N_CORES)))
    return np.concatenate(
        [res.results[i]["out"] for i in range(N_CORES)], axis=0
    ).astype(np.float32)
